# revision 43
# baseline (speedup 1.0000x reference)
"""BiLSTM-CRF loss kernel for 8 Trainium2 NeuronCores.

Phase 1 (LSTM + emissions): 8 cores = 2 directions x 4 batch-quarters
(16 examples/core). Each example's 512 steps are split into 8 chains of
64 steps; chains 1-7 start BURN steps early from zero state (the LSTM
forgets its initial state well inside BURN steps) and the first BURN
outputs are discarded. All 8 chains x 16 examples = 128 columns advance
in lockstep, so every recurrent matmul streams a 128-wide rhs — 4x the
arithmetic per weight load vs a 32-wide layout, and the dense matmul
bursts keep the PE out of the HAM half-clock throttle. Per merged step:
4 bias matmuls (K=8 indicator trick, start=True clears the psum bank),
32 wih matmuls (input projection), 64 whh matmuls ordered f,i -> g -> o
so the gating chain (sigmoid f,i -> tanh g -> cell update -> tanh c ->
h) overlaps the tail of the burst. Emissions are computed every EGRP
steps as a column-packed GEMM (4 T=32 strips via tile_position share
one PSUM bank, borrowing a slot from the fi psum ring).

Phase 2 (CRF partition function): exp-space linear recurrence
a' = (M^T a) * exp(em_t), split into 32 chunks (16 forward + 16
backward) of 16 steps, stacked 4-per-128-partitions in eight stacks.
All stacks share the same block-diagonal transition matrix, so they
ride as columns of a single [128,128]x[128,64] matmul + one [128,64]
DVE multiply per iteration -- 24 iterations instead of 511 sequential
steps. Chunks 2..16 of each direction start from a uniform state 8
steps early: the CRF direction converges at ~0.1/step (transition
logits are +-0.1), and the per-example scale offsets are recovered
exactly on the host by telescoping sum-ratio snapshots taken at
iterations 8 and 15. Renormalization (every 8 iters, tracked in log
space) is computed off the critical path and applied as an extra
multiply in the following iteration.
"""

import numpy as np
import ml_dtypes

import concourse.bacc as bacc
import concourse.mybir as mybir
from concourse import tile
from concourse.bass_utils import run_bass_kernel_spmd

V, T, E, H = 50000, 32, 256, 512
B, S = 64, 512
BC = 16            # examples per core (phase 1)
B2 = 8             # examples per core (phase 2)
NCORES = 8
RENORM = 8         # CRF renormalization cadence

AF = mybir.ActivationFunctionType
F32 = mybir.dt.float32
BF16 = mybir.dt.bfloat16
ALU = mybir.AluOpType

CH = 8             # chains per example
CL = S // CH       # steps per chain (64)
BURN = 6           # burn-in steps for chains 1..CH-1
ITERS = CL + BURN  # merged iteration count
W = CH * BC        # merged batch width = 128
CHUNK = 10         # steps per embedding-DMA chunk
EGRP = 10          # steps per emission GEMM generation
ECOL = EGRP * W // 4   # emission psum columns per strip (320)

# psum gate-block order f,i,o,g ; PyTorch row order is i,f,g,o
GPERM = np.r_[512:1024, 0:512, 1536:2048, 1024:1536]

_built = {}


def _new_nc():
    return bacc.Bacc("TRN2", target_bir_lowering=False, debug=False,
                     num_devices=NCORES)


def build_phase1(iters=ITERS):
    nc = _new_nc()
    ng = iters // EGRP                # emission generations
    nch = iters // CHUNK
    xpd = nc.dram_tensor("xpd", [128, 16, iters * W], BF16,
                         kind="ExternalInput")
    whh = nc.dram_tensor("whhb", [128, 4, 4 * H], BF16, kind="ExternalInput")
    fcw = nc.dram_tensor("fcwb", [128, 4, T], BF16, kind="ExternalInput")
    idd = nc.dram_tensor("idd", [128, 128], BF16, kind="ExternalInput")
    emo = nc.dram_tensor("emo", [128, ng * ECOL], F32, kind="ExternalOutput")

    with tile.TileContext(nc) as tc:
        with (
            tc.tile_pool(name="weights", bufs=1) as wpool,
            tc.tile_pool(name="state", bufs=1) as spool,
            tc.tile_pool(name="et", bufs=2) as epool,
            tc.tile_pool(name="gact", bufs=2) as apool,
            tc.tile_pool(name="pp", bufs=2) as ppool,
            tc.tile_pool(name="tch", bufs=2) as tpool,
            tc.tile_pool(name="est", bufs=2) as espool,
            tc.tile_pool(name="psfi", bufs=2, space="PSUM") as pfip,
            tc.tile_pool(name="psog", bufs=2, space="PSUM") as pogp,
        ):
            whh_s = wpool.tile([128, 4, 4 * H], BF16, tag="whh")
            fcw_s = wpool.tile([128, 4, T], BF16, tag="fcw")
            id_s = wpool.tile([128, 128], BF16, tag="ident")
            hbuf = spool.tile([128, 4, iters * W], BF16, tag="hbuf")
            cg = spool.tile([128, 8, W], BF16, tag="cg")  # [c | g~] merged

            def et_dma(ch):
                etile = epool.tile([128, 16, CHUNK * W], BF16, tag="xq")
                cs = slice(ch * CHUNK * W, (ch + 1) * CHUNK * W)
                nc.gpsimd.dma_start(etile[:], xpd[:, :, cs])
                return etile

            # DMA issue order tracks first use: priming needs the identity
            # and the first xp chunk immediately; whh only at step 1 and
            # fcw at the first emission generation
            nc.gpsimd.dma_start(id_s[:], idd[:, :])
            et_tiles = [et_dma(0)]
            nc.gpsimd.dma_start(whh_s[:], whh[:, :, :])
            et_tiles.append(et_dma(1))
            nc.gpsimd.dma_start(fcw_s[:], fcw[:, :, :])
            nc.vector.memset(cg[:, 0:4, :], 0.0)

            def prime(t, half):
                """Inject the host-precomputed input projection + bias
                (2 identity matmuls, start=True clears the banks) for
                merged step t into a fresh fi (half=0) or og (half=1)
                psum tile."""
                pool = pfip if half == 0 else pogp
                tag = "fi" if half == 0 else "og"
                tl = pool.tile([128, 8, W], F32, tag=tag)
                et = et_tiles[(t // CHUNK) % 2]
                es = slice((t % CHUNK) * W, (t % CHUNK + 1) * W)
                for nb in range(2):
                    nc.tensor.matmul(
                        tl[:, nb * 4:(nb + 1) * 4, :], id_s[:],
                        et[:, half * 8 + nb * 4:half * 8 + (nb + 1) * 4, es],
                        start=True, stop=False, skip_group_check=True)
                return tl

            fi_cur = prime(0, 0)
            og_cur = prime(0, 1)
            fi_nxt = prime(1, 0)
            og_nxt = prime(1, 1)

            def whh_burst(dst, t, mlist):
                # k-outer so the first sub-burst only needs hbuf k-block 0,
                # which the split h-write below makes available first
                hs = slice((t - 1) * W, t * W)
                for k in range(4):
                    for m in mlist:
                        nc.tensor.matmul(
                            dst[:, m % 8, :],
                            whh_s[:, k, m * 128:(m + 1) * 128],
                            hbuf[:, k, hs], start=False, stop=False,
                            skip_group_check=True)

            for t in range(iters):
                if t > 0:
                    whh_burst(fi_cur, t, range(8))          # f,i blocks
                    whh_burst(og_cur, t, [12, 13, 14, 15])  # g blocks
                    whh_burst(og_cur, t, [8, 9, 10, 11])    # o blocks
                gfi = apool.tile([128, 8, W], BF16, tag="gfi")
                nc.scalar.activation(gfi[:], fi_cur[:], AF.Sigmoid)
                nc.scalar.activation(cg[:, 4:8, :], og_cur[:, 4:8, :],
                                     AF.Tanh)
                pp = ppool.tile([128, 8, W], BF16, tag="pp")
                nc.vector.tensor_mul(pp[:], gfi[:], cg[:])
                nc.vector.tensor_add(cg[:, 0:4, :], pp[:, 0:4, :],
                                     pp[:, 4:8, :])
                go = apool.tile([128, 4, W], BF16, tag="go")
                nc.scalar.activation(go[:], og_cur[:, 0:4, :], AF.Sigmoid)
                tch = tpool.tile([128, 4, W], BF16, tag="tch")
                nc.scalar.activation(tch[:], cg[:, 0:4, :], AF.Tanh)
                for k in range(4):
                    nc.vector.tensor_mul(
                        hbuf[:, k, t * W:(t + 1) * W],
                        go[:, k, :], tch[:, k, :])
                fi_cur, og_cur = fi_nxt, og_nxt
                if (t + 1) % EGRP == 0:
                    # emission GEMM for the last EGRP steps; borrows a slot
                    # from the fi psum ring (4 T=32 col-strips share a bank)
                    g = (t + 1) // EGRP - 1
                    pe = pfip.tile([128, ECOL], F32, tag="fi")
                    for k in range(4):
                        for s in range(4):
                            cs = slice((4 * g + s) * ECOL,
                                       (4 * g + s + 1) * ECOL)
                            nc.tensor.matmul(
                                pe[32 * s:32 * (s + 1), :], fcw_s[:, k, :],
                                hbuf[:, k, cs],
                                start=(k == 0 and s == 0),
                                stop=(k == 3 and s == 3),
                                skip_group_check=True,
                                tile_position=(0, 32 * s))
                    est = espool.tile([128, ECOL], F32, tag="est")
                    nc.scalar.copy(est[:], pe[:])
                    nc.gpsimd.dma_start(emo[:, g * ECOL:(g + 1) * ECOL],
                                        est[:])
                if t + 2 < iters:
                    fi_nxt = prime(t + 2, 0)
                    og_nxt = prime(t + 2, 1)
                if t % CHUNK == CHUNK - 2 and t // CHUNK + 2 < nch:
                    et_tiles[(t // CHUNK) % 2] = et_dma(t // CHUNK + 2)
    nc.compile()
    return nc


def build_phase2(nsteps=S):
    """Stacked CRF: forward alpha chain (partitions 0:32) and backward
    beta chain (32:64) advance together: one [64,64]x[64,8] matmul + one
    [64,8] DVE multiply per iteration. Renormalization (every RENORM
    iters) is computed off the critical path from the unnormalized state
    and applied as an extra multiply in the next iteration."""
    half = nsteps // 2                 # 256 merged iterations
    nc = _new_nc()
    em2 = nc.dram_tensor("emS2", [64, half * B2], F32, kind="ExternalInput")
    bdd = nc.dram_tensor("bdiag", [64, 66], BF16, kind="ExternalInput")
    s2d = nc.dram_tensor("sel2", [2, 64], F32, kind="ExternalInput")
    vO = nc.dram_tensor("vO", [64, B2], BF16, kind="ExternalOutput")
    bO = nc.dram_tensor("bO", [64, B2], F32, kind="ExternalOutput")
    zO = nc.dram_tensor("zO", [2, B2], F32, kind="ExternalOutput")

    with tile.TileContext(nc) as tc:
        with (
            tc.tile_pool(name="sb", bufs=1) as sb,
            tc.tile_pool(name="ab", bufs=4) as ab,
            tc.tile_pool(name="rr", bufs=2) as rr,
            tc.tile_pool(name="pm", bufs=4, space="PSUM") as pm,
            tc.tile_pool(name="pr", bufs=2, space="PSUM") as pr,
        ):
            em_s = sb.tile([64, half * B2], F32, tag="em")
            emx = sb.tile([64, half * B2], BF16, tag="emx")
            bd_s = sb.tile([64, 66], BF16, tag="bd")   # blockdiag | side-sel
            s2_s = sb.tile([2, 64], F32, tag="s2")     # side broadcast
            z_s = sb.tile([2, B2], F32, tag="z")
            nc.gpsimd.dma_start(em_s[:], em2[:, :])
            nc.gpsimd.dma_start(bd_s[:], bdd[:, :])
            nc.gpsimd.dma_start(s2_s[0:2, :], s2d[:, :])
            nc.vector.memset(z_s[:], 0.0)
            nc.scalar.activation(emx[:], em_s[:], AF.Exp)

            def exslice(i):
                return emx[:, i * B2:(i + 1) * B2]

            v = ab.tile([64, B2], BF16, tag="v")
            nc.vector.tensor_scalar_add(v[:], exslice(0), 0.0)
            rb_pending = None

            for i in range(1, half):
                pt = pm.tile([64, B2], F32, tag="pt")
                nc.tensor.matmul(pt[:], bd_s[:, 0:64], v[:],
                                 start=True, stop=True)
                if rb_pending is not None:
                    vr = ab.tile([64, B2], BF16, tag="v")
                    nc.vector.tensor_mul(vr[:], pt[:], rb_pending[:])
                    v2 = ab.tile([64, B2], BF16, tag="v")
                    nc.vector.tensor_mul(v2[:], vr[:], exslice(i))
                    rb_pending = None
                else:
                    v2 = ab.tile([64, B2], BF16, tag="v")
                    nc.vector.tensor_mul(v2[:], pt[:], exslice(i))
                v = v2
                if i % RENORM == RENORM - 1 and i < half - 1:
                    # side sums of the unnormalized state (off critical path)
                    ps2 = pr.tile([2, B2], F32, tag="ps2")
                    nc.tensor.matmul(ps2[:], bd_s[:, 64:66], v[:],
                                     start=True, stop=True)
                    rec = rr.tile([2, B2], F32, tag="rec")
                    nc.vector.reciprocal(rec[:], ps2[:])
                    rbt = pr.tile([64, B2], F32, tag="rbt")
                    nc.tensor.matmul(rbt[:], s2_s[0:2, :], rec[:],
                                     start=True, stop=True)
                    rb_sb = rr.tile([64, B2], F32, tag="rbs")
                    nc.vector.tensor_scalar_add(rb_sb[:], rbt[:], 0.0)
                    rb_pending = rb_sb
                    lg = rr.tile([2, B2], F32, tag="lg")
                    nc.scalar.activation(lg[:], ps2[:], AF.Ln)
                    nc.vector.tensor_add(z_s[:], z_s[:], lg[:])

            # final: beta half-step (bout = mb.T @ u_256); v holds
            # [a_255 ; u_256]
            ptf = pm.tile([64, B2], F32, tag="pt")
            nc.tensor.matmul(ptf[:], bd_s[:, 0:64], v[:],
                             start=True, stop=True)
            bout = sb.tile([64, B2], F32, tag="bout")
            nc.vector.tensor_scalar_add(bout[:], ptf[:], 0.0)
            nc.gpsimd.dma_start(bO[:, :], bout[:])
            nc.gpsimd.dma_start(vO[:, :], v[:])
            nc.gpsimd.dma_start(zO[:, :], z_s[:])
    nc.compile()
    return nc


def _bf16(a):
    return np.ascontiguousarray(np.asarray(a).astype(ml_dtypes.bfloat16))


def _prep_core_p1(e_core, wih_d, whh_d, b_d, fcw_half):
    """e_core: [16, S, E] bf16 embeddings (already reversed for bwd).
    Builds the chain-interleaved input: merged iteration i, column
    c*16+e holds example e of chain c at global step start_c + i, where
    start_c = 0 for chain 0 and c*CL - BURN otherwise."""
    et = e_core.transpose(1, 0, 2)          # [S, BC, E]
    ET = np.empty((ITERS, W, E), np.float32)
    for c in range(CH):
        s0 = 0 if c == 0 else c * CL - BURN
        ET[:, c * BC:(c + 1) * BC, :] = et[s0:s0 + ITERS]
    wp = wih_d[GPERM]                       # [4H, E]
    bp = b_d[GPERM]                         # [4H]
    # host-side input projection: xp = e @ wih.T + b, laid out to match
    # the gate psum [128 part, 16 m-blocks, cols]
    xp = ET.reshape(ITERS * W, E) @ wp.T + bp
    xpb = np.ascontiguousarray(
        xp.T.reshape(16, 128, ITERS * W).transpose(1, 0, 2).astype(
            ml_dtypes.bfloat16))
    hp = whh_d[GPERM]                       # [4H, H]
    whhb = np.ascontiguousarray(
        hp.T.reshape(4, 128, 4 * H).transpose(1, 0, 2).astype(
            ml_dtypes.bfloat16))
    fcwb = np.ascontiguousarray(
        fcw_half.T.reshape(4, 128, T).transpose(1, 0, 2).astype(
            ml_dtypes.bfloat16))
    ident = np.ascontiguousarray(np.eye(128, dtype=ml_dtypes.bfloat16))
    return {"xpd": xpb, "whhb": whhb, "fcwb": fcwb, "idd": ident}


def _deinterleave_em(emo):
    """emo: [128, NG*ECOL] device output -> [S, BC, T] emissions."""
    ng = ITERS // EGRP
    emT = emo.reshape(4, 32, ng, ECOL).transpose(1, 2, 0, 3).reshape(
        T, ITERS * W)
    r2 = emT.reshape(T, ITERS, CH, BC)
    out = np.empty((S, BC, T), np.float32)
    for c in range(CH):
        i0 = 0 if c == 0 else BURN
        out[c * CL:(c + 1) * CL] = r2[:, i0:i0 + CL, c, :].transpose(1, 2, 0)
    return out


def kernel(emb, w_ih_f, w_hh_f, b_f, w_ih_b, w_hh_b, b_b, fc_w, fc_b,
           start_trans, end_trans, trans, x, tags):
    emb = np.asarray(emb, np.float32)
    fc_w = np.asarray(fc_w, np.float32)
    fc_b = np.asarray(fc_b, np.float32)
    start_trans = np.asarray(start_trans, np.float32)
    end_trans = np.asarray(end_trans, np.float32)
    trans = np.asarray(trans, np.float32)
    x = np.asarray(x).astype(np.int64)
    tags_np = np.asarray(tags).astype(np.int64)

    if "p1" not in _built:
        _built["p1"] = build_phase1()
        _built["p2"] = build_phase2()
    nc1, nc2 = _built["p1"], _built["p2"]

    embb = emb.astype(ml_dtypes.bfloat16)
    in_maps = []
    for core in range(NCORES):
        d = core // 4          # 0 = forward, 1 = backward
        q = core % 4
        xs = x[q * BC:(q + 1) * BC]
        if d == 1:
            xs = xs[:, ::-1]
        ec = embb[xs]          # [16, S, E] bf16
        if d == 0:
            in_maps.append(_prep_core_p1(
                ec, np.asarray(w_ih_f, np.float32),
                np.asarray(w_hh_f, np.float32),
                np.asarray(b_f, np.float32), fc_w[:, :H]))
        else:
            in_maps.append(_prep_core_p1(
                ec, np.asarray(w_ih_b, np.float32),
                np.asarray(w_hh_b, np.float32),
                np.asarray(b_b, np.float32), fc_w[:, H:]))
    r1 = run_bass_kernel_spmd(nc1, in_maps, core_ids=list(range(NCORES)))

    em = np.empty((S, B, T), np.float32)
    for q in range(4):
        emf = _deinterleave_em(r1.results[q]["emo"])
        emb_r = _deinterleave_em(r1.results[4 + q]["emo"])
        em[:, q * BC:(q + 1) * BC, :] = emf + emb_r[::-1] + fc_b
    em[0] += start_trans

    # gold-path (numerator) score; start_trans already folded into em[0]
    tags_t = tags_np.T
    emit = np.take_along_axis(em, tags_t[:, :, None], axis=2)[..., 0].sum(0)
    tr = trans[tags_t[:-1], tags_t[1:]].sum(0)
    num = emit + tr + end_trans[tags_t[-1]]

    Mx = np.exp(trans.astype(np.float64))
    bd = np.zeros((128, 132), np.float32)
    for c in range(4):
        blk = Mx if c < 2 else Mx.T
        bd[c * T:(c + 1) * T, c * T:(c + 1) * T] = blk
        bd[c * T:(c + 1) * T, 128 + c] = 1.0
    sel4 = np.zeros((4, 128), np.float32)
    for c in range(4):
        sel4[c, c * T:(c + 1) * T] = 1.0
    # chain k of a direction lives in stack (k-1)//2, block (k-1)%2 (fwd)
    # or 2+(k-1)%2 (bwd); chunk k covers 16 steps ending at 16k-1 (fwd)
    # / starting at 512-16k (bwd), with an 8-step burn-in prefix
    def fmap(k):
        return (lambda j: j) if k == 1 else (lambda j: 16 * k - 25 + j)

    def bmap(k):
        return (lambda j: S - 1 - j) if k == 1 else \
            (lambda j: 536 - 16 * k - j)

    def pos(k, bwd):
        return ((k - 1) // 2, (2 if bwd else 0) + (k - 1) % 2)

    in_maps2 = []
    for core in range(NCORES):
        emc = em[:, core * B2:(core + 1) * B2, :]           # [S, 8, T]
        ef = emc.transpose(2, 0, 1)                         # [T, S, 8]
        emS4 = np.zeros((128, IT2 + 1, 8, B2), np.float32)
        emS4[0:32, 0, 0] = ef[:, 0]
        emS4[64:96, 0, 0] = ef[:, S - 1] + end_trans[:, None]
        js = range(1, IT2 + 1)
        for k in range(1, 17):
            for bwd in (0, 1):
                tb, c = pos(k, bwd)
                f = bmap(k) if bwd else fmap(k)
                emS4[c * 32:(c + 1) * 32, 1:, tb] = \
                    ef[:, [min(f(j), S - 1) for j in js]]
        emS4 = np.ascontiguousarray(
            emS4.reshape(128, (IT2 + 1) * 8 * B2))
        in_maps2.append({"emS4": emS4, "bdiag4": _bf16(bd), "sel4": sel4})
    r2 = run_bass_kernel_spmd(nc2, in_maps2, core_ids=list(range(NCORES)))

    den = np.empty(B, np.float64)
    for core in range(NCORES):
        vf = r2.results[core]["vO"].astype(np.float64)      # [128, 32]
        sv = r2.results[core]["svO"].astype(np.float64)     # [128, 64]
        zo = r2.results[core]["zO"].astype(np.float64)      # [4, 96]
        # sv cols: [j8 all stacks | j15 all stacks]; zo: [j8 | j15 | final]
        WC = 8 * B2
        V = {8: sv[:, 0:WC], 15: sv[:, WC:2 * WC], 'f': vf}
        Z = {8: zo[:, 0:WC], 15: zo[:, WC:2 * WC], 'f': zo[:, 2 * WC:3 * WC]}

        def lr(snap, tb, c):
            vv = V[snap][c * 32:(c + 1) * 32, tb * B2:(tb + 1) * B2]
            return np.log(vv.sum(0)) + Z[snap][c, tb * B2:(tb + 1) * B2]

        logc = sum(lr(8, *pos(k, 0)) - lr(15 if k == 2 else 'f',
                                          *pos(k - 1, 0))
                   for k in range(2, 17))
        logd = sum(lr(8, *pos(k, 1)) - lr(15 if k == 2 else 'f',
                                          *pos(k - 1, 1))
                   for k in range(2, 17))
        tbF, cF = pos(16, 0)
        tbB, cB = pos(16, 1)
        F8 = vf[cF * 32:(cF + 1) * 32, tbF * B2:(tbF + 1) * B2]
        B8 = vf[cB * 32:(cB + 1) * 32, tbB * B2:(tbB + 1) * B2]
        den[core * B2:(core + 1) * B2] = (
            np.log((F8 * (Mx @ B8)).sum(0))
            + Z['f'][cF, tbF * B2:(tbF + 1) * B2]
            + Z['f'][cB, tbB * B2:(tbB + 1) * B2] - logc - logd)

    llh = num - den
    return np.float32(-llh.mean())


# revision 44
# speedup vs baseline: 1.0084x; 1.0084x over previous
"""BiLSTM-CRF loss kernel for 8 Trainium2 NeuronCores.

Phase 1 (LSTM + emissions): 8 cores = 2 directions x 4 batch-quarters
(16 examples/core). Each example's 512 steps are split into 8 chains of
64 steps; chains 1-7 start BURN steps early from zero state (the LSTM
forgets its initial state well inside BURN steps) and the first BURN
outputs are discarded. All 8 chains x 16 examples = 128 columns advance
in lockstep, so every recurrent matmul streams a 128-wide rhs — 4x the
arithmetic per weight load vs a 32-wide layout, and the dense matmul
bursts keep the PE out of the HAM half-clock throttle. Per merged step:
4 bias matmuls (K=8 indicator trick, start=True clears the psum bank),
32 wih matmuls (input projection), 64 whh matmuls ordered f,i -> g -> o
so the gating chain (sigmoid f,i -> tanh g -> cell update -> tanh c ->
h) overlaps the tail of the burst. Emissions are computed every EGRP
steps as a column-packed GEMM (4 T=32 strips via tile_position share
one PSUM bank, borrowing a slot from the fi psum ring).

Phase 2 (CRF partition function): exp-space linear recurrence
a' = (M^T a) * exp(em_t), split into 32 chunks (16 forward + 16
backward) of 16 steps, stacked 4-per-128-partitions in eight stacks.
All stacks share the same block-diagonal transition matrix, so they
ride as columns of a single [128,128]x[128,64] matmul + one [128,64]
DVE multiply per iteration -- 24 iterations instead of 511 sequential
steps. Chunks 2..16 of each direction start from a uniform state 8
steps early: the CRF direction converges at ~0.1/step (transition
logits are +-0.1), and the per-example scale offsets are recovered
exactly on the host by telescoping sum-ratio snapshots taken at
iterations 8 and 15. Renormalization (every 8 iters, tracked in log
space) is computed off the critical path and applied as an extra
multiply in the following iteration.
"""

import numpy as np
import ml_dtypes

import concourse.bacc as bacc
import concourse.mybir as mybir
from concourse import tile
from concourse.bass_utils import run_bass_kernel_spmd

V, T, E, H = 50000, 32, 256, 512
B, S = 64, 512
BC = 16            # examples per core (phase 1)
B2 = 8             # examples per core (phase 2)
NCORES = 8
RENORM = 8         # CRF renormalization cadence

AF = mybir.ActivationFunctionType
F32 = mybir.dt.float32
BF16 = mybir.dt.bfloat16
ALU = mybir.AluOpType

CH = 8             # chains per example
CL = S // CH       # steps per chain (64)
BURN = 6           # burn-in steps for chains 1..CH-1
ITERS = CL + BURN  # merged iteration count
W = CH * BC        # merged batch width = 128
CHUNK = 10         # steps per embedding-DMA chunk
EGRP = 10          # steps per emission GEMM generation
ECOL = EGRP * W // 4   # emission psum columns per strip (320)

# psum gate-block order f,i,o,g ; PyTorch row order is i,f,g,o
GPERM = np.r_[512:1024, 0:512, 1536:2048, 1024:1536]

_built = {}


def _new_nc():
    return bacc.Bacc("TRN2", target_bir_lowering=False, debug=False,
                     num_devices=NCORES)


def build_phase1(iters=ITERS):
    nc = _new_nc()
    ng = iters // EGRP                # emission generations
    nch = iters // CHUNK
    eb = nc.dram_tensor("eb", [128, 2, iters * W], BF16, kind="ExternalInput")
    wih = nc.dram_tensor("wihb", [128, 2, 4 * H], BF16, kind="ExternalInput")
    whh = nc.dram_tensor("whhb", [128, 4, 4 * H], BF16, kind="ExternalInput")
    fcw = nc.dram_tensor("fcwb", [128, 4, T], BF16, kind="ExternalInput")
    mscd = nc.dram_tensor("miscd", [8, 10, 128], BF16, kind="ExternalInput")
    emo = nc.dram_tensor("emo", [128, ng * ECOL], F32, kind="ExternalOutput")

    with tile.TileContext(nc) as tc:
        with (
            tc.tile_pool(name="weights", bufs=1) as wpool,
            tc.tile_pool(name="state", bufs=1) as spool,
            tc.tile_pool(name="et", bufs=2) as epool,
            tc.tile_pool(name="gact", bufs=2) as apool,
            tc.tile_pool(name="pp", bufs=2) as ppool,
            tc.tile_pool(name="tch", bufs=2) as tpool,
            tc.tile_pool(name="est", bufs=2) as espool,
            tc.tile_pool(name="psfi", bufs=2, space="PSUM") as pfip,
            tc.tile_pool(name="psog", bufs=2, space="PSUM") as pogp,
        ):
            wih_s = wpool.tile([128, 2, 4 * H], BF16, tag="wih")
            whh_s = wpool.tile([128, 4, 4 * H], BF16, tag="whh")
            fcw_s = wpool.tile([128, 4, T], BF16, tag="fcw")
            msc_s = wpool.tile([8, 10, 128], BF16, tag="msc")
            hbuf = spool.tile([128, 4, iters * W], BF16, tag="hbuf")
            cg = spool.tile([128, 8, W], BF16, tag="cg")  # [c | g~] merged

            def et_dma(ch):
                etile = epool.tile([128, 2, CHUNK * W], BF16, tag="et")
                cs = slice(ch * CHUNK * W, (ch + 1) * CHUNK * W)
                nc.gpsimd.dma_start(etile[:], eb[:, :, cs])
                return etile

            # DMA issue order tracks first use: priming needs misc, the
            # first embedding chunk and wih immediately; whh only at step
            # 1 and fcw at the first emission generation
            nc.gpsimd.dma_start(msc_s[0:8, :, :], mscd[:, :, :])
            et_tiles = [et_dma(0)]
            nc.gpsimd.dma_start(wih_s[:], wih[:, :, :])
            et_tiles.append(et_dma(1))
            nc.gpsimd.dma_start(whh_s[:], whh[:, :, :])
            nc.gpsimd.dma_start(fcw_s[:], fcw[:, :, :])
            nc.vector.memset(cg[:, 0:4, :], 0.0)

            def prime(t, half):
                """Bias (2 start=True indicator matmuls) + input projection
                (16 MMs) for merged step t into a fresh fi (half=0) or og
                (half=1) psum tile. All-PE priming avoids cross-engine
                head-of-line stalls in the PE queue."""
                pool = pfip if half == 0 else pogp
                tag = "fi" if half == 0 else "og"
                tl = pool.tile([128, 8, W], F32, tag=tag)
                for nb in range(2):
                    nc.tensor.matmul(
                        tl[:, nb * 4:(nb + 1) * 4, :],
                        msc_s[0:8, half, :],
                        msc_s[0:8, 2 + nb * 4:2 + (nb + 1) * 4, :],
                        start=True, stop=False, skip_group_check=True)
                et = et_tiles[(t // CHUNK) % 2]
                es = slice((t % CHUNK) * W, (t % CHUNK + 1) * W)
                for mb in range(8):
                    m = half * 8 + mb
                    for k in range(2):
                        nc.tensor.matmul(
                            tl[:, mb, :], wih_s[:, k, m * 128:(m + 1) * 128],
                            et[:, k, es], start=False, stop=False,
                            skip_group_check=True)
                return tl

            fi_cur = prime(0, 0)
            og_cur = prime(0, 1)
            fi_nxt = prime(1, 0)
            og_nxt = prime(1, 1)

            def whh_burst(dst, t, mlist):
                # k-outer so the first sub-burst only needs hbuf k-block 0,
                # which the split h-write below makes available first
                hs = slice((t - 1) * W, t * W)
                for k in range(4):
                    for m in mlist:
                        nc.tensor.matmul(
                            dst[:, m % 8, :],
                            whh_s[:, k, m * 128:(m + 1) * 128],
                            hbuf[:, k, hs], start=False, stop=False,
                            skip_group_check=True)

            for t in range(iters):
                if t > 0:
                    whh_burst(fi_cur, t, range(8))          # f,i blocks
                    whh_burst(og_cur, t, [12, 13, 14, 15])  # g blocks
                    whh_burst(og_cur, t, [8, 9, 10, 11])    # o blocks
                gfi = apool.tile([128, 8, W], BF16, tag="gfi")
                nc.scalar.activation(gfi[:], fi_cur[:], AF.Sigmoid)
                nc.scalar.activation(cg[:, 4:8, :], og_cur[:, 4:8, :],
                                     AF.Tanh)
                pp = ppool.tile([128, 8, W], BF16, tag="pp")
                nc.vector.tensor_mul(pp[:], gfi[:], cg[:])
                nc.vector.tensor_add(cg[:, 0:4, :], pp[:, 0:4, :],
                                     pp[:, 4:8, :])
                go = apool.tile([128, 4, W], BF16, tag="go")
                nc.scalar.activation(go[:], og_cur[:, 0:4, :], AF.Sigmoid)
                tch = tpool.tile([128, 4, W], BF16, tag="tch")
                nc.scalar.activation(tch[:], cg[:, 0:4, :], AF.Tanh)
                for k in range(4):
                    nc.vector.tensor_mul(
                        hbuf[:, k, t * W:(t + 1) * W],
                        go[:, k, :], tch[:, k, :])
                fi_cur, og_cur = fi_nxt, og_nxt
                if (t + 1) % EGRP == 0:
                    # emission GEMM for the last EGRP steps; borrows a slot
                    # from the fi psum ring (4 T=32 col-strips share a bank)
                    g = (t + 1) // EGRP - 1
                    pe = pfip.tile([128, ECOL], F32, tag="fi")
                    for k in range(4):
                        for s in range(4):
                            cs = slice((4 * g + s) * ECOL,
                                       (4 * g + s + 1) * ECOL)
                            nc.tensor.matmul(
                                pe[32 * s:32 * (s + 1), :], fcw_s[:, k, :],
                                hbuf[:, k, cs],
                                start=(k == 0 and s == 0),
                                stop=(k == 3 and s == 3),
                                skip_group_check=True,
                                tile_position=(0, 32 * s))
                    est = espool.tile([128, ECOL], F32, tag="est")
                    nc.scalar.copy(est[:], pe[:])
                    nc.gpsimd.dma_start(emo[:, g * ECOL:(g + 1) * ECOL],
                                        est[:])
                if t + 2 < iters:
                    fi_nxt = prime(t + 2, 0)
                    og_nxt = prime(t + 2, 1)
                if t % CHUNK == CHUNK - 2 and t // CHUNK + 2 < nch:
                    et_tiles[(t // CHUNK) % 2] = et_dma(t // CHUNK + 2)
    nc.compile()
    return nc


def build_phase2(nsteps=S):
    """Stacked CRF: forward alpha chain (partitions 0:32) and backward
    beta chain (32:64) advance together: one [64,64]x[64,8] matmul + one
    [64,8] DVE multiply per iteration. Renormalization (every RENORM
    iters) is computed off the critical path from the unnormalized state
    and applied as an extra multiply in the next iteration."""
    half = nsteps // 2                 # 256 merged iterations
    nc = _new_nc()
    em2 = nc.dram_tensor("emS2", [64, half * B2], F32, kind="ExternalInput")
    bdd = nc.dram_tensor("bdiag", [64, 66], BF16, kind="ExternalInput")
    s2d = nc.dram_tensor("sel2", [2, 64], F32, kind="ExternalInput")
    vO = nc.dram_tensor("vO", [64, B2], BF16, kind="ExternalOutput")
    bO = nc.dram_tensor("bO", [64, B2], F32, kind="ExternalOutput")
    zO = nc.dram_tensor("zO", [2, B2], F32, kind="ExternalOutput")

    with tile.TileContext(nc) as tc:
        with (
            tc.tile_pool(name="sb", bufs=1) as sb,
            tc.tile_pool(name="ab", bufs=4) as ab,
            tc.tile_pool(name="rr", bufs=2) as rr,
            tc.tile_pool(name="pm", bufs=4, space="PSUM") as pm,
            tc.tile_pool(name="pr", bufs=2, space="PSUM") as pr,
        ):
            em_s = sb.tile([64, half * B2], F32, tag="em")
            emx = sb.tile([64, half * B2], BF16, tag="emx")
            bd_s = sb.tile([64, 66], BF16, tag="bd")   # blockdiag | side-sel
            s2_s = sb.tile([2, 64], F32, tag="s2")     # side broadcast
            z_s = sb.tile([2, B2], F32, tag="z")
            nc.gpsimd.dma_start(em_s[:], em2[:, :])
            nc.gpsimd.dma_start(bd_s[:], bdd[:, :])
            nc.gpsimd.dma_start(s2_s[0:2, :], s2d[:, :])
            nc.vector.memset(z_s[:], 0.0)
            nc.scalar.activation(emx[:], em_s[:], AF.Exp)

            def exslice(i):
                return emx[:, i * B2:(i + 1) * B2]

            v = ab.tile([64, B2], BF16, tag="v")
            nc.vector.tensor_scalar_add(v[:], exslice(0), 0.0)
            rb_pending = None

            for i in range(1, half):
                pt = pm.tile([64, B2], F32, tag="pt")
                nc.tensor.matmul(pt[:], bd_s[:, 0:64], v[:],
                                 start=True, stop=True)
                if rb_pending is not None:
                    vr = ab.tile([64, B2], BF16, tag="v")
                    nc.vector.tensor_mul(vr[:], pt[:], rb_pending[:])
                    v2 = ab.tile([64, B2], BF16, tag="v")
                    nc.vector.tensor_mul(v2[:], vr[:], exslice(i))
                    rb_pending = None
                else:
                    v2 = ab.tile([64, B2], BF16, tag="v")
                    nc.vector.tensor_mul(v2[:], pt[:], exslice(i))
                v = v2
                if i % RENORM == RENORM - 1 and i < half - 1:
                    # side sums of the unnormalized state (off critical path)
                    ps2 = pr.tile([2, B2], F32, tag="ps2")
                    nc.tensor.matmul(ps2[:], bd_s[:, 64:66], v[:],
                                     start=True, stop=True)
                    rec = rr.tile([2, B2], F32, tag="rec")
                    nc.vector.reciprocal(rec[:], ps2[:])
                    rbt = pr.tile([64, B2], F32, tag="rbt")
                    nc.tensor.matmul(rbt[:], s2_s[0:2, :], rec[:],
                                     start=True, stop=True)
                    rb_sb = rr.tile([64, B2], F32, tag="rbs")
                    nc.vector.tensor_scalar_add(rb_sb[:], rbt[:], 0.0)
                    rb_pending = rb_sb
                    lg = rr.tile([2, B2], F32, tag="lg")
                    nc.scalar.activation(lg[:], ps2[:], AF.Ln)
                    nc.vector.tensor_add(z_s[:], z_s[:], lg[:])

            # final: beta half-step (bout = mb.T @ u_256); v holds
            # [a_255 ; u_256]
            ptf = pm.tile([64, B2], F32, tag="pt")
            nc.tensor.matmul(ptf[:], bd_s[:, 0:64], v[:],
                             start=True, stop=True)
            bout = sb.tile([64, B2], F32, tag="bout")
            nc.vector.tensor_scalar_add(bout[:], ptf[:], 0.0)
            nc.gpsimd.dma_start(bO[:, :], bout[:])
            nc.gpsimd.dma_start(vO[:, :], v[:])
            nc.gpsimd.dma_start(zO[:, :], z_s[:])
    nc.compile()
    return nc


def _bf16(a):
    return np.ascontiguousarray(np.asarray(a).astype(ml_dtypes.bfloat16))


def _prep_core_p1(e_core, wih_d, whh_d, b_d, fcw_half):
    """e_core: [16, S, E] bf16 embeddings (already reversed for bwd).
    Builds the chain-interleaved input: merged iteration i, column
    c*16+e holds example e of chain c at global step start_c + i, where
    start_c = 0 for chain 0 and c*CL - BURN otherwise."""
    et = e_core.transpose(1, 0, 2)          # [S, BC, E]
    ET = np.empty((ITERS, W, E), et.dtype)
    for c in range(CH):
        s0 = 0 if c == 0 else c * CL - BURN
        ET[:, c * BC:(c + 1) * BC, :] = et[s0:s0 + ITERS]
    eT = np.ascontiguousarray(
        ET.transpose(2, 0, 1).reshape(2, 128, ITERS * W)
        .transpose(1, 0, 2))                # [128, 2, ITERS*W]
    wp = wih_d[GPERM]                       # [4H, E]
    wihb = np.ascontiguousarray(
        wp.T.reshape(2, 128, 4 * H).transpose(1, 0, 2).astype(
            ml_dtypes.bfloat16))
    hp = whh_d[GPERM]                       # [4H, H]
    whhb = np.ascontiguousarray(
        hp.T.reshape(4, 128, 4 * H).transpose(1, 0, 2).astype(
            ml_dtypes.bfloat16))
    fcwb = np.ascontiguousarray(
        fcw_half.T.reshape(4, 128, T).transpose(1, 0, 2).astype(
            ml_dtypes.bfloat16))
    bp = b_d[GPERM]                         # [4H]
    misc = np.zeros((8, 10, 128), ml_dtypes.bfloat16)
    misc[:, 0:2, :] = bp.reshape(2, 8, 128).transpose(1, 0, 2)
    for j in range(8):
        misc[j, 2 + j, :] = 1.0
    return {"eb": eT, "wihb": wihb, "whhb": whhb, "fcwb": fcwb,
            "miscd": misc}


def _deinterleave_em(emo):
    """emo: [128, NG*ECOL] device output -> [S, BC, T] emissions."""
    ng = ITERS // EGRP
    emT = emo.reshape(4, 32, ng, ECOL).transpose(1, 2, 0, 3).reshape(
        T, ITERS * W)
    r2 = emT.reshape(T, ITERS, CH, BC)
    out = np.empty((S, BC, T), np.float32)
    for c in range(CH):
        i0 = 0 if c == 0 else BURN
        out[c * CL:(c + 1) * CL] = r2[:, i0:i0 + CL, c, :].transpose(1, 2, 0)
    return out


def kernel(emb, w_ih_f, w_hh_f, b_f, w_ih_b, w_hh_b, b_b, fc_w, fc_b,
           start_trans, end_trans, trans, x, tags):
    emb = np.asarray(emb, np.float32)
    fc_w = np.asarray(fc_w, np.float32)
    fc_b = np.asarray(fc_b, np.float32)
    start_trans = np.asarray(start_trans, np.float32)
    end_trans = np.asarray(end_trans, np.float32)
    trans = np.asarray(trans, np.float32)
    x = np.asarray(x).astype(np.int64)
    tags_np = np.asarray(tags).astype(np.int64)

    if "p1" not in _built:
        _built["p1"] = build_phase1()
        _built["p2"] = build_phase2()
    nc1, nc2 = _built["p1"], _built["p2"]

    embb = emb.astype(ml_dtypes.bfloat16)
    in_maps = []
    for core in range(NCORES):
        d = core // 4          # 0 = forward, 1 = backward
        q = core % 4
        xs = x[q * BC:(q + 1) * BC]
        if d == 1:
            xs = xs[:, ::-1]
        ec = embb[xs]          # [16, S, E] bf16
        if d == 0:
            in_maps.append(_prep_core_p1(
                ec, np.asarray(w_ih_f, np.float32),
                np.asarray(w_hh_f, np.float32),
                np.asarray(b_f, np.float32), fc_w[:, :H]))
        else:
            in_maps.append(_prep_core_p1(
                ec, np.asarray(w_ih_b, np.float32),
                np.asarray(w_hh_b, np.float32),
                np.asarray(b_b, np.float32), fc_w[:, H:]))
    r1 = run_bass_kernel_spmd(nc1, in_maps, core_ids=list(range(NCORES)))

    em = np.empty((S, B, T), np.float32)
    for q in range(4):
        emf = _deinterleave_em(r1.results[q]["emo"])
        emb_r = _deinterleave_em(r1.results[4 + q]["emo"])
        em[:, q * BC:(q + 1) * BC, :] = emf + emb_r[::-1] + fc_b
    em[0] += start_trans

    # gold-path (numerator) score; start_trans already folded into em[0]
    tags_t = tags_np.T
    emit = np.take_along_axis(em, tags_t[:, :, None], axis=2)[..., 0].sum(0)
    tr = trans[tags_t[:-1], tags_t[1:]].sum(0)
    num = emit + tr + end_trans[tags_t[-1]]

    Mx = np.exp(trans.astype(np.float64))
    bd = np.zeros((128, 132), np.float32)
    for c in range(4):
        blk = Mx if c < 2 else Mx.T
        bd[c * T:(c + 1) * T, c * T:(c + 1) * T] = blk
        bd[c * T:(c + 1) * T, 128 + c] = 1.0
    sel4 = np.zeros((4, 128), np.float32)
    for c in range(4):
        sel4[c, c * T:(c + 1) * T] = 1.0
    # chain k of a direction lives in stack (k-1)//2, block (k-1)%2 (fwd)
    # or 2+(k-1)%2 (bwd); chunk k covers 16 steps ending at 16k-1 (fwd)
    # / starting at 512-16k (bwd), with an 8-step burn-in prefix
    def fmap(k):
        return (lambda j: j) if k == 1 else (lambda j: 16 * k - 25 + j)

    def bmap(k):
        return (lambda j: S - 1 - j) if k == 1 else \
            (lambda j: 536 - 16 * k - j)

    def pos(k, bwd):
        return ((k - 1) // 2, (2 if bwd else 0) + (k - 1) % 2)

    in_maps2 = []
    for core in range(NCORES):
        emc = em[:, core * B2:(core + 1) * B2, :]           # [S, 8, T]
        ef = emc.transpose(2, 0, 1)                         # [T, S, 8]
        emS4 = np.zeros((128, IT2 + 1, 8, B2), np.float32)
        emS4[0:32, 0, 0] = ef[:, 0]
        emS4[64:96, 0, 0] = ef[:, S - 1] + end_trans[:, None]
        js = range(1, IT2 + 1)
        for k in range(1, 17):
            for bwd in (0, 1):
                tb, c = pos(k, bwd)
                f = bmap(k) if bwd else fmap(k)
                emS4[c * 32:(c + 1) * 32, 1:, tb] = \
                    ef[:, [min(f(j), S - 1) for j in js]]
        emS4 = np.ascontiguousarray(
            emS4.reshape(128, (IT2 + 1) * 8 * B2))
        in_maps2.append({"emS4": emS4, "bdiag4": _bf16(bd), "sel4": sel4})
    r2 = run_bass_kernel_spmd(nc2, in_maps2, core_ids=list(range(NCORES)))

    den = np.empty(B, np.float64)
    for core in range(NCORES):
        vf = r2.results[core]["vO"].astype(np.float64)      # [128, 32]
        sv = r2.results[core]["svO"].astype(np.float64)     # [128, 64]
        zo = r2.results[core]["zO"].astype(np.float64)      # [4, 96]
        # sv cols: [j8 all stacks | j15 all stacks]; zo: [j8 | j15 | final]
        WC = 8 * B2
        V = {8: sv[:, 0:WC], 15: sv[:, WC:2 * WC], 'f': vf}
        Z = {8: zo[:, 0:WC], 15: zo[:, WC:2 * WC], 'f': zo[:, 2 * WC:3 * WC]}

        def lr(snap, tb, c):
            vv = V[snap][c * 32:(c + 1) * 32, tb * B2:(tb + 1) * B2]
            return np.log(vv.sum(0)) + Z[snap][c, tb * B2:(tb + 1) * B2]

        logc = sum(lr(8, *pos(k, 0)) - lr(15 if k == 2 else 'f',
                                          *pos(k - 1, 0))
                   for k in range(2, 17))
        logd = sum(lr(8, *pos(k, 1)) - lr(15 if k == 2 else 'f',
                                          *pos(k - 1, 1))
                   for k in range(2, 17))
        tbF, cF = pos(16, 0)
        tbB, cB = pos(16, 1)
        F8 = vf[cF * 32:(cF + 1) * 32, tbF * B2:(tbF + 1) * B2]
        B8 = vf[cB * 32:(cB + 1) * 32, tbB * B2:(tbB + 1) * B2]
        den[core * B2:(core + 1) * B2] = (
            np.log((F8 * (Mx @ B8)).sum(0))
            + Z['f'][cF, tbF * B2:(tbF + 1) * B2]
            + Z['f'][cB, tbB * B2:(tbB + 1) * B2] - logc - logd)

    llh = num - den
    return np.float32(-llh.mean())
